# revision 5
# baseline (speedup 1.0000x reference)
"""HOA-to-binaural FIR convolution on 8 Trainium2 NeuronCores.

y[b,e,t] = sum_c sum_k f[e,c,k] * x[b,c,t-k]   (causal, zero-padded past)

Strategy: data-parallel over batch (1 batch per core). On each core the
convolution is computed on the TensorEngine as a sequence of accumulating
matmuls using a Toeplitz expansion of the (small) filter:

  out[tau, n] += sum_p  R_ec[p, 128d + tau] * X_c[p, n0 + n - d]

where X_c[p, n] = x[c, 128n + p] (block-interleaved input, built on host
in bf16) and R_ec[p, j] = f[e, c, j - p] (shifted-filter Toeplitz buffer,
built on host in bf16, ~17.4 MB). out[tau, n] = y[e, 128*(n0+n) + tau]
accumulates over channels c (16) and tap-blocks d (17) in PSUM (fp32).

Per core: 8 time-groups x 2 ears x 272 matmuls of [K=128, M=128, N=512].
"""

import numpy as np
import ml_dtypes

P = 128
B, C, T = 8, 16, 480000
E, TAPS = 2, 2048

_cache = {}


def _build_nc(nb, ng, c, taps, evict="dvet"):
    import sys
    if "/opt/trn_rl_repo" not in sys.path:
        sys.path.insert(0, "/opt/trn_rl_repo")
    import concourse.tile as tile
    from concourse import bacc, mybir

    d = taps // P + 1
    halo = d - 1
    jw = taps + P
    ngroups = -(-nb // ng)
    nk = -(-ng // P)  # 128-chunks per group

    nc = bacc.Bacc("TRN2", target_bir_lowering=False, debug=False, num_devices=8)
    xb = nc.dram_tensor("xb", [P, c, halo + nb], mybir.dt.bfloat16, kind="ExternalInput")
    rb = nc.dram_tensor("rb", [P, c, E, jw], mybir.dt.bfloat16, kind="ExternalInput")
    y = nc.dram_tensor("y", [E, nb, P], mybir.dt.float32, kind="ExternalOutput")

    with tile.TileContext(nc) as tc:
        with (
            tc.tile_pool(name="rpool", bufs=1) as rpool,
            tc.tile_pool(name="xpool", bufs=2) as xpool,
            tc.tile_pool(name="opool", bufs=4) as opool,
            tc.tile_pool(name="psum", bufs=4, space="PSUM") as psum,
        ):
            rts = []
            for ci in range(c):
                rt = rpool.tile([P, E, jw], mybir.dt.bfloat16, tag=f"r{ci}")
                nc.sync.dma_start(rt[:], rb[:, ci])
                rts.append(rt)
            for g in range(ngroups):
                n0 = g * ng
                n = min(ng, nb - n0)
                xg = xpool.tile([P, c, ng + halo], mybir.dt.bfloat16, tag="xg")
                step = max(1, c // 4)
                for c0 in range(0, c, step):
                    c1 = min(c0 + step, c)
                    nc.sync.dma_start(
                        xg[:, c0:c1, : n + halo], xb[:, c0:c1, n0 : n0 + n + halo]
                    )
                for e in range(E):
                    acc = psum.tile([P, ng], mybir.dt.float32, tag="acc")
                    for ci in range(c):
                        for di in range(d):
                            nc.tensor.matmul(
                                acc[:, :n],
                                rts[ci][:, e, di * P : (di + 1) * P],
                                xg[:, ci, halo - di : halo - di + n],
                                start=(ci == 0 and di == 0),
                                stop=(ci == c - 1 and di == d - 1),
                            )
                    if evict == "strided":
                        ot = opool.tile([P, ng], mybir.dt.float32, tag="ot")
                        nc.vector.tensor_copy(ot[:, :n], acc[:, :n])
                        nc.sync.dma_start(
                            y[e, n0 : n0 + n, :].rearrange("n p -> p n"), ot[:, :n]
                        )
                        continue
                    # dvet: full 128x128 transposes via DVE 32x32 blocks,
                    # then fully-contiguous DRAM stores.
                    asb = opool.tile([P, -(-ng // 32) * 32], mybir.dt.float32, tag="asb")
                    osb = opool.tile([P, nk, P], mybir.dt.float32, tag="osb")
                    nc.vector.tensor_copy(asb[:, :n], acc[:, :n])
                    n32 = -(-n // 32) * 32
                    if n32 > n:
                        nc.vector.memset(asb[:, n:n32], 0.0)
                    for k2 in range(-(-n // P)):
                        na = min(P, n32 - k2 * P)  # cols of this chunk (mult of 32)
                        for a in range(na // 32):
                            for b in range(P // 32):
                                nc.vector.transpose(
                                    osb[32 * a : 32 * a + 32, k2, 32 * b : 32 * b + 32],
                                    asb[32 * b : 32 * b + 32,
                                        k2 * P + 32 * a : k2 * P + 32 * a + 32],
                                )
                    if n % P == 0:
                        nc.sync.dma_start(
                            y[e, n0 : n0 + n, :].rearrange("(k q) p -> q k p", q=P),
                            osb[:],
                        )
                    else:
                        for k2 in range(-(-n // P)):
                            qn = min(P, n - k2 * P)
                            nc.sync.dma_start(
                                y[e, n0 + k2 * P : n0 + k2 * P + qn, :],
                                osb[:qn, k2, :],
                            )
    nc.compile()
    return nc


def _prep_filter(filt, taps):
    """filt [E, c, taps] fp32 -> R [P, c, E, taps+P] bf16 with
    R[p, ci, e, j] = filt[e, ci, j - p] (zero outside tap range)."""
    e_, c, _ = filt.shape
    jw = taps + P
    fb = filt.astype(ml_dtypes.bfloat16)
    r = np.zeros((P, c, e_, jw), dtype=ml_dtypes.bfloat16)
    ft = np.ascontiguousarray(fb.transpose(1, 0, 2))  # [c, E, taps]
    for p in range(P):
        r[p, :, :, p : p + taps] = ft
    return r


def _prep_x(x, halo):
    """x [c, T] fp32 -> Xh [P, c, halo + nb] bf16 with
    Xh[p, ci, halo + n] = x[ci, 128n + p]; leading halo blocks zero."""
    c, t = x.shape
    nb = t // P
    xi = x.astype(ml_dtypes.bfloat16).reshape(c, nb, P).transpose(2, 0, 1)
    xh = np.zeros((P, c, halo + nb), dtype=ml_dtypes.bfloat16)
    xh[:, :, halo:] = xi
    return xh


def _run(hoa, filt, nb, ng, c, taps, trace=False, **kw):
    from concourse.bass_utils import run_bass_kernel_spmd

    key = (nb, ng, c, taps)
    if key not in _cache:
        _cache[key] = _build_nc(nb, ng, c, taps)
    nc = _cache[key]

    halo = taps // P
    r = _prep_filter(filt, taps)
    n_cores = hoa.shape[0]
    in_maps = [{"xb": _prep_x(hoa[b], halo), "rb": r} for b in range(n_cores)]
    res = run_bass_kernel_spmd(nc, in_maps, list(range(n_cores)), trace=trace, **kw)
    out = np.stack([res.results[i]["y"] for i in range(n_cores)])
    return out.reshape(n_cores, E, nb * P), res


def kernel(hoa, hoa2bin_t):
    hoa = np.asarray(hoa, dtype=np.float32)
    filt = np.asarray(hoa2bin_t, dtype=np.float32)
    assert hoa.shape == (B, C, T) and filt.shape == (E, C, TAPS)
    return _run(hoa, filt, T // P, 512, C, TAPS)[0].astype(np.float32)


# revision 8
# speedup vs baseline: 1.0250x; 1.0250x over previous
"""HOA-to-binaural FIR convolution on 8 Trainium2 NeuronCores.

y[b,e,t] = sum_c sum_k f[e,c,k] * x[b,c,t-k]   (causal, zero-padded past)

Strategy: data-parallel over batch (1 batch per core). On each core the
convolution is computed on the TensorEngine as a sequence of accumulating
matmuls using a Toeplitz expansion of the (small) filter:

  out[tau, n] += sum_p  R_ec[p, 128d + tau] * X_c[p, n0 + n - d]

where X_c[p, n] = x[c, 128n + p] (block-interleaved input, built on host
in bf16) and R_ec[p, j] = f[e, c, j - p] (shifted-filter Toeplitz buffer,
built on host in bf16, ~17.4 MB). out[tau, n] = y[e, 128*(n0+n) + tau]
accumulates over channels c (16) and tap-blocks d (17) in PSUM (fp32).

Per core: 8 time-groups x 2 ears x 272 matmuls of [K=128, M=128, N=512].
"""

import numpy as np
import ml_dtypes

P = 128
B, C, T = 8, 16, 480000
E, TAPS = 2, 2048

_cache = {}


def _build_nc(nb, ng, c, taps, evict="dvet"):
    import sys
    if "/opt/trn_rl_repo" not in sys.path:
        sys.path.insert(0, "/opt/trn_rl_repo")
    import concourse.tile as tile
    from concourse import bacc, mybir

    d = taps // P + 1
    halo = d - 1
    jw = taps + P
    ngroups = -(-nb // ng)
    nk = -(-ng // P)  # 128-chunks per group

    nc = bacc.Bacc("TRN2", target_bir_lowering=False, debug=False, num_devices=8)
    xb = nc.dram_tensor("xb", [P, c, halo + nb], mybir.dt.bfloat16, kind="ExternalInput")
    rb = nc.dram_tensor("rb", [P, c, E, jw], mybir.dt.bfloat16, kind="ExternalInput")
    y = nc.dram_tensor("y", [E, nb, P], mybir.dt.float32, kind="ExternalOutput")

    with tile.TileContext(nc) as tc:
        with (
            tc.tile_pool(name="rpool", bufs=1) as rpool,
            tc.tile_pool(name="xpool", bufs=2) as xpool,
            tc.tile_pool(name="opool", bufs=4) as opool,
            tc.tile_pool(name="psum", bufs=4, space="PSUM") as psum,
        ):
            def load_group(g):
                n0 = g * ng
                n = min(ng, nb - n0)
                xg = xpool.tile([P, c, ng + halo], mybir.dt.bfloat16, tag="xg")
                step = max(1, c // 4)
                for c0 in range(0, c, step):
                    c1 = min(c0 + step, c)
                    nc.sync.dma_start(
                        xg[:, c0:c1, : n + halo], xb[:, c0:c1, n0 : n0 + n + halo]
                    )
                return xg

            # group-0 input load FIRST so its DMAs aren't queued behind the
            # 17.8MB Toeplitz-filter load (measured 53us PE startup stall).
            xg0 = load_group(0)
            rts = []
            for ci in range(c):
                rt = rpool.tile([P, E, jw], mybir.dt.bfloat16, tag=f"r{ci}")
                nc.sync.dma_start(rt[:], rb[:, ci])
                rts.append(rt)
            for g in range(ngroups):
                n0 = g * ng
                n = min(ng, nb - n0)
                xg = xg0 if g == 0 else load_group(g)
                for e in range(E):
                    acc = psum.tile([P, ng], mybir.dt.float32, tag="acc")
                    for ci in range(c):
                        for di in range(d):
                            nc.tensor.matmul(
                                acc[:, :n],
                                rts[ci][:, e, di * P : (di + 1) * P],
                                xg[:, ci, halo - di : halo - di + n],
                                start=(ci == 0 and di == 0),
                                stop=(ci == c - 1 and di == d - 1),
                            )
                    if evict == "strided":
                        ot = opool.tile([P, ng], mybir.dt.float32, tag="ot")
                        nc.vector.tensor_copy(ot[:, :n], acc[:, :n])
                        nc.sync.dma_start(
                            y[e, n0 : n0 + n, :].rearrange("n p -> p n"), ot[:, :n]
                        )
                        continue
                    # dvet: full 128x128 transposes via DVE 32x32 blocks,
                    # then fully-contiguous DRAM stores.
                    asb = opool.tile([P, -(-ng // 32) * 32], mybir.dt.float32, tag="asb")
                    osb = opool.tile([P, nk, P], mybir.dt.float32, tag="osb")
                    nc.vector.tensor_copy(asb[:, :n], acc[:, :n])
                    n32 = -(-n // 32) * 32
                    if n32 > n:
                        nc.vector.memset(asb[:, n:n32], 0.0)
                    for k2 in range(-(-n // P)):
                        na = min(P, n32 - k2 * P)  # cols of this chunk (mult of 32)
                        for a in range(na // 32):
                            for b in range(P // 32):
                                nc.vector.transpose(
                                    osb[32 * a : 32 * a + 32, k2, 32 * b : 32 * b + 32],
                                    asb[32 * b : 32 * b + 32,
                                        k2 * P + 32 * a : k2 * P + 32 * a + 32],
                                )
                    if n % P == 0:
                        nc.sync.dma_start(
                            y[e, n0 : n0 + n, :].rearrange("(k q) p -> q k p", q=P),
                            osb[:],
                        )
                    else:
                        for k2 in range(-(-n // P)):
                            qn = min(P, n - k2 * P)
                            nc.sync.dma_start(
                                y[e, n0 + k2 * P : n0 + k2 * P + qn, :],
                                osb[:qn, k2, :],
                            )
    nc.compile()
    return nc


def _prep_filter(filt, taps):
    """filt [E, c, taps] fp32 -> R [P, c, E, taps+P] bf16 with
    R[p, ci, e, j] = filt[e, ci, j - p] (zero outside tap range)."""
    e_, c, _ = filt.shape
    jw = taps + P
    fb = filt.astype(ml_dtypes.bfloat16)
    r = np.zeros((P, c, e_, jw), dtype=ml_dtypes.bfloat16)
    ft = np.ascontiguousarray(fb.transpose(1, 0, 2))  # [c, E, taps]
    for p in range(P):
        r[p, :, :, p : p + taps] = ft
    return r


def _prep_x(x, halo):
    """x [c, T] fp32 -> Xh [P, c, halo + nb] bf16 with
    Xh[p, ci, halo + n] = x[ci, 128n + p]; leading halo blocks zero."""
    c, t = x.shape
    nb = t // P
    xi = x.astype(ml_dtypes.bfloat16).reshape(c, nb, P).transpose(2, 0, 1)
    xh = np.zeros((P, c, halo + nb), dtype=ml_dtypes.bfloat16)
    xh[:, :, halo:] = xi
    return xh


def _run(hoa, filt, nb, ng, c, taps, trace=False, **kw):
    from concourse.bass_utils import run_bass_kernel_spmd

    key = (nb, ng, c, taps)
    if key not in _cache:
        _cache[key] = _build_nc(nb, ng, c, taps)
    nc = _cache[key]

    halo = taps // P
    r = _prep_filter(filt, taps)
    n_cores = hoa.shape[0]
    in_maps = [{"xb": _prep_x(hoa[b], halo), "rb": r} for b in range(n_cores)]
    res = run_bass_kernel_spmd(nc, in_maps, list(range(n_cores)), trace=trace, **kw)
    out = np.stack([res.results[i]["y"] for i in range(n_cores)])
    return out.reshape(n_cores, E, nb * P), res


def _axon_ok():
    try:
        import jax
        return sum(1 for d in jax.devices() if "cpu" not in d.platform.lower()) >= B
    except Exception:
        return False


def _kernel_subprocess(hoa, filt):
    """Fallback: the calling process has no axon jax backend (e.g. it pinned
    JAX_PLATFORMS=cpu to run the reference). Re-run in a clean subprocess."""
    import os, subprocess, sys, tempfile
    me = os.path.abspath(__file__)
    with tempfile.TemporaryDirectory() as td:
        np.savez(os.path.join(td, "in.npz"), hoa=hoa, filt=filt)
        env = dict(os.environ)
        env.pop("JAX_PLATFORMS", None)
        env["JAX_PLATFORMS"] = "axon"
        site = "/root/.axon_site"
        pp = [site, f"{site}/_ro/trn_rl_repo", f"{site}/_ro/pypackages"]
        env["PYTHONPATH"] = ":".join(pp)
        code = (
            "import numpy as np, importlib.util, os\n"
            f"spec = importlib.util.spec_from_file_location('knl', {me!r})\n"
            "m = importlib.util.module_from_spec(spec); spec.loader.exec_module(m)\n"
            f"d = np.load(os.path.join({td!r}, 'in.npz'))\n"
            "y = m.kernel(d['hoa'], d['filt'])\n"
            f"np.save(os.path.join({td!r}, 'out.npy'), y)\n"
        )
        subprocess.run([sys.executable, "-c", code], env=env, check=True)
        return np.load(os.path.join(td, "out.npy"))


def kernel(hoa, hoa2bin_t):
    hoa = np.asarray(hoa, dtype=np.float32)
    filt = np.asarray(hoa2bin_t, dtype=np.float32)
    assert hoa.shape == (B, C, T) and filt.shape == (E, C, TAPS)
    if not _axon_ok():
        return _kernel_subprocess(hoa, filt)
    return _run(hoa, filt, T // P, 469, C, TAPS)[0].astype(np.float32)
